# revision 6
# baseline (speedup 1.0000x reference)
"""Trainium2 Bass kernel for CandidateFinder (retrieval_knn).

Math: for each (batch, query row), candidates = the K_MAX=64 smallest key
indices whose 32-dim sign pattern matches the query's in either dim-group
(dims 0:32 or 32:64), ascending, padded with -1.  This equals the
reference's per-group-topk + merge (per-group truncation can never drop an
index that would make the merged top-64).

Host side (sharding/layout only, no arithmetic): batch b and query-half
go to core 2b+half; only the high byte of each f32 is shipped (sign +
7 exponent bits; byte<128 <=> x>0 whenever |x| >= 2^-125, true for this
data) laid out dim-major: pairs of 128-row query slabs stacked into the
four 32-partition PE strips, keys replicated onto the upper 64
partitions, so four K=32 matmuls run concurrently in PE row-groups.

Device per core:
  - two consolidated u8 DMAs (SP: queries+key-chunk0, ACT: chunks 1-3),
    4x less HBM traffic than f32; DVE quantizes to {+0.5,-0.5} bf16 per
    512-col chunk in the DMA shadow (ns-outer loop order)
  - all-pairs group dots (match <=> dot == 8) in [128,1024] PSUM tiles;
    detection: 16 DVE row-max + 16 ACT relu-sum units, threshold 7.9
  - global any-match flag via ones-matmul partition reduce -> register;
    fast path ships all -1 early (gpsimd queue); slow path re-DMAs
  - rare exact path (tc.If): recompute dots, build (-index / -4096)
    values, extract 64 smallest via vector.max + match_replace
"""

import numpy as np

import concourse.bacc as bacc
import concourse.mybir as mybir
from concourse.tile import TileContext
from concourse import bass_utils, bass_isa

B, L, D = 4, 2048, 64
HALF = 1024          # query rows per core
N_CORES = 8
K_MAX = 64
G = 32               # dims per group
QT = HALF // 128     # 8 query slabs per core
MATCH_DOT = 8.0      # 32 * 0.5 * 0.5
THRESH = 7.9         # between 7.75 (best non-match) and 8.0
SENT = 4096.0        # sentinel > any index

f32 = mybir.dt.float32
bf16 = mybir.dt.bfloat16
i32 = mybir.dt.int32
u8 = mybir.dt.uint8
u32 = mybir.dt.uint32
Alu = mybir.AluOpType
Ax = mybir.AxisListType
AF = mybir.ActivationFunctionType

_CACHE = {}


def _build():
    nc = bacc.Bacc("TRN2", target_bir_lowering=False,
                   enable_partition_id=False)
    # inb0[:, 0:512]  = qb4[h*64+d, pair*128+p] = sign byte of
    #                   q[p*8 + 2*pair + h, d]
    # inb0[:, 512:1024] / inb1 = kb4[dup*64+d, j] = sign byte of k[j, d]
    inb0 = nc.dram_tensor("inb0", [128, 1024], u8, kind="ExternalInput")
    inb1 = nc.dram_tensor("inb1", [128, 1536], u8, kind="ExternalInput")
    out = nc.dram_tensor("out", [HALF, K_MAX], i32, kind="ExternalOutput")
    out_pt = out[:].rearrange("(p t) c -> p (t c)", p=128)

    with TileContext(nc) as tc:
        with tc.tile_pool(name="sb", bufs=1) as sb, \
             tc.tile_pool(name="sb2", bufs=3) as sb2, \
             tc.tile_pool(name="ps", bufs=2, space="PSUM") as ps:

            # ---- load sign bytes (two kicks); quantize on DVE ----
            ib0 = sb.tile([128, 1024], u8)
            ib1 = sb.tile([128, 1536], u8)
            sqT4 = sb.tile([128, HALF // 2], bf16)
            skT4 = sb.tile([128, L], bf16)
            nc.default_dma_engine.dma_start(ib0[:, 0:512], inb0[:, 0:512])
            nc.scalar.dma_start(ib1, inb1[:, :])
            nc.default_dma_engine.dma_start(ib0[:, 512:1024],
                                            inb0[:, 512:1024])
            # byte < 128 <=> sign bit clear <=> x > 0  ->  +0.5 else -0.5
            nc.vector.tensor_scalar(sqT4, ib0[:, 0:512], 128.0, 0.5,
                                    op0=Alu.is_lt, op1=Alu.subtract)
            nc.vector.tensor_scalar(skT4[:, 0:512], ib0[:, 512:1024],
                                    128.0, 0.5,
                                    op0=Alu.is_lt, op1=Alu.subtract)
            for c in range(3):
                cs = slice(c * 512, (c + 1) * 512)
                nc.vector.tensor_scalar(skT4[:, 512 + c * 512:
                                             1024 + c * 512],
                                        ib1[:, cs], 128.0, 0.5,
                                        op0=Alu.is_lt, op1=Alu.subtract)

            # ---- early fast-path output: all -1 (gpsimd memset + kick) ----
            out_sb = sb.tile([128, QT * K_MAX], i32)
            nc.gpsimd.memset(out_sb, -1)
            nc.gpsimd.dma_start(out_pt, out_sb)

            # ---- all-pairs dots, 4 concurrent K=32 matmuls ----
            # rstat col semantics: >= THRESH iff any match (DVE row-max
            # cols top at 8; ACT relu-sum cols are 0 or >= 8)
            rstat = sb.tile([128, 32], f32)
            rbias = sb.tile([128, 1], f32)
            nc.vector.memset(rbias, -80.0 * THRESH)
            for ns in range(4):
                kc = slice(ns * 512, (ns + 1) * 512)
                for pair in range(QT // 2):
                    it = ns * 4 + pair
                    qc = slice(pair * 128, (pair + 1) * 128)
                    pG0 = ps.tile([128, 1024], f32, tag="g0")
                    pG1 = ps.tile([128, 1024], f32, tag="g1")
                    nc.tensor.matmul(pG0[:, 0:512], lhsT=sqT4[0:32, qc],
                                     rhs=skT4[0:32, kc], start=True,
                                     stop=True, tile_position=(0, 0))
                    nc.tensor.matmul(pG1[:, 0:512], lhsT=sqT4[32:64, qc],
                                     rhs=skT4[32:64, kc], start=True,
                                     stop=True, tile_position=(32, 0))
                    nc.tensor.matmul(pG0[:, 512:1024], lhsT=sqT4[64:96, qc],
                                     rhs=skT4[64:96, kc], start=True,
                                     stop=True, tile_position=(64, 0))
                    nc.tensor.matmul(pG1[:, 512:1024], lhsT=sqT4[96:128, qc],
                                     rhs=skT4[96:128, kc], start=True,
                                     stop=True, tile_position=(96, 0))
                    # detection: g0 -> DVE row-max, g1 -> ACT relu-sum
                    nc.vector.tensor_reduce(
                        rstat[:, 2 * it:2 * it + 1], pG0,
                        axis=Ax.X, op=Alu.max)
                    scr = sb2.tile([128, 1024], bf16, tag="scr")
                    nc.scalar.activation(
                        scr, pG1, AF.Relu, bias=rbias[:, 0:1],
                        scale=80.0,
                        accum_out=rstat[:, 2 * it + 1:2 * it + 2])

            # ---- scalar any-match flag ----
            ones = sb.tile([128, 1], f32)
            nc.vector.memset(ones, 1.0)
            sr = sb.tile([128, 1], f32)
            nc.vector.tensor_reduce(sr, rstat, axis=Ax.X, op=Alu.max)
            srf = sb.tile([128, 1], f32)
            nc.vector.tensor_scalar(srf, sr, THRESH, None, op0=Alu.is_ge)
            fps = ps.tile([1, 1], f32, tag="g0")
            nc.tensor.matmul(fps, lhsT=ones, rhs=srf, start=True, stop=True)
            flag = sb.tile([1, 1], i32)
            nc.vector.tensor_scalar(flag, fps[0:1, 0:1], 0.5, None,
                                    op0=Alu.is_ge)
            rv = nc.values_load(
                flag[0:1, 0:1], min_val=0, max_val=1,
                skip_runtime_bounds_check=True,
                engines=(mybir.EngineType.PE, mybir.EngineType.DVE,
                         mybir.EngineType.Pool, mybir.EngineType.SP))

            # ---- rare exact path (recompute + extract, then re-DMA) ----
            with tc.If(rv > 0):
                c2i = sb.tile([128, L], i32)   # SENT - j (key j = column)
                nc.gpsimd.iota(c2i, pattern=[[-1, L]], base=int(SENT),
                               channel_multiplier=0)
                c2f = sb.tile([128, L], f32)
                nc.gpsimd.tensor_copy(c2f, c2i)
                negone = sb.tile([128, K_MAX], f32)
                nc.vector.memset(negone, -1.0)
                for t in range(QT):
                    base = (t % 2) * 64
                    qc = slice((t // 2) * 128, (t // 2) * 128 + 128)
                    lhs0 = sqT4[base:base + 32, qc]
                    lhs1 = sqT4[base + 32:base + 64, qc]
                    val = sb.tile([128, L], f32, tag="val")
                    for h in range(2):
                        p0 = ps.tile([128, 1024], f32, tag="g0")
                        p1 = ps.tile([128, 1024], f32, tag="g1")
                        for s in range(2):
                            kc = slice(h * 1024 + s * 512,
                                       h * 1024 + (s + 1) * 512)
                            sl = slice(s * 512, (s + 1) * 512)
                            nc.tensor.matmul(p0[:, sl], lhsT=lhs0,
                                             rhs=skT4[base:base + 32, kc],
                                             start=True, stop=True,
                                             tile_position=(base, 0))
                            nc.tensor.matmul(p1[:, sl], lhsT=lhs1,
                                             rhs=skT4[base + 32:base + 64,
                                                      kc],
                                             start=True, stop=True,
                                             tile_position=(base + 32, 0))
                        hsl = slice(h * 1024, (h + 1) * 1024)
                        m0 = sb2.tile([128, 1024], f32, tag="m0")
                        nc.vector.tensor_scalar(m0, p0, THRESH,
                                                None, op0=Alu.is_ge)
                        m1 = sb2.tile([128, 1024], f32, tag="m1")
                        nc.vector.scalar_tensor_tensor(
                            m1, in0=p1, scalar=THRESH, in1=m0,
                            op0=Alu.is_ge, op1=Alu.max)
                        # val = m1 ? -(j) : -SENT  ==  m1*(SENT-j) - SENT
                        nc.vector.tensor_tensor(
                            out=val[:, hsl], in0=m1, in1=c2f[:, hsl],
                            op=Alu.mult)
                        nc.vector.tensor_scalar_add(val[:, hsl], val[:, hsl],
                                                    -SENT)
                    # 64 smallest j == 64 largest of val, descending
                    no = sb.tile([128, K_MAX], f32, tag="no")
                    for it8 in range(8):
                        osl = slice(it8 * 8, (it8 + 1) * 8)
                        nc.vector.max(out=no[:, osl], in_=val)
                        nc.vector.match_replace(
                            out=val, in_to_replace=no[:, osl],
                            in_values=val, imm_value=-SENT)
                    jv = sb.tile([128, K_MAX], f32, tag="jv")
                    nc.vector.tensor_scalar_mul(jv, no, -1.0)  # j or SENT
                    msk = sb.tile([128, K_MAX], u32, tag="msk")
                    nc.vector.tensor_scalar(msk, jv, 2048.5, None,
                                            op0=Alu.is_ge)
                    nc.vector.copy_predicated(jv, msk, negone)
                    nc.vector.tensor_copy(
                        out_sb[:, t * K_MAX:(t + 1) * K_MAX], jv)
                # overwrite the early -1s with the exact candidates
                nc.default_dma_engine.dma_start(out_pt, out_sb)

    nc.compile()
    return nc


def get_nc():
    if "nc" not in _CACHE:
        _CACHE["nc"] = _build()
    return _CACHE["nc"]


def _sign_bytes(x):
    """High byte of each f32 (pure byte-level layout transform)."""
    x = np.ascontiguousarray(x, dtype=np.float32)
    return x.view(np.uint8).reshape(x.shape + (4,))[..., 3]


def make_in_maps(query_up, key_up):
    """Pure layout transforms (byte-slice/transpose/replicate) per core."""
    qb = _sign_bytes(np.asarray(query_up, dtype=np.float32))  # [B, L, D] u8
    kbb = _sign_bytes(np.asarray(key_up, dtype=np.float32))   # [B, L, D] u8
    in_maps = []
    for c in range(N_CORES):
        b, half = c // 2, c % 2
        q = qb[b, half * HALF:(half + 1) * HALF]             # [1024, 64] u8
        # [p, pair, h, d] -> [h, d, pair, p] -> [128, 512]
        qb4 = np.ascontiguousarray(
            q.reshape(128, 4, 2, D).transpose(2, 3, 1, 0).reshape(
                128, HALF // 2))
        kT = kbb[b].T                                        # [64, 2048] u8
        kb4 = np.concatenate([kT, kT], axis=0)               # [128, 2048]
        inb0 = np.ascontiguousarray(
            np.concatenate([qb4, kb4[:, 0:512]], axis=1))    # [128, 1024]
        inb1 = np.ascontiguousarray(kb4[:, 512:2048])        # [128, 1536]
        in_maps.append({"inb0": inb0, "inb1": inb1})
    return in_maps


def kernel(query_up, key_up, head_idx=None, **_ignored):
    nc = get_nc()
    in_maps = make_in_maps(query_up, key_up)
    res = bass_utils.run_bass_kernel_spmd(
        nc, in_maps, core_ids=list(range(N_CORES)))
    full = np.empty((B, L, K_MAX), dtype=np.int32)
    for c in range(N_CORES):
        b, half = c // 2, c % 2
        # out row p*8 + t <-> query row p*8 + 2*(t//2) + t%2 == p*8 + t
        full[b, half * HALF:(half + 1) * HALF] = res.results[c]["out"]
    return full
